# revision 6
# baseline (speedup 1.0000x reference)
"""Trainium2 Bass kernel for nn_EquiLinearLayer.

Computes  out[s,n,j,y] = sum_{i,x,b} weights[j,i,b] * blade[b,x,y] * x[s,n,i,x]
for x:[8,2048,512,16] f32, weights:[512,512,9] f32, blade:[9,16,16] f32.

Strategy (all-TensorE, two matmul phases, data-parallel over points):
  * 16384 points are split across 8 cores (2048 each), grouped in 8s
    (dpt in 0..7), 256 groups per core.
  * Host preps:
      xT[g, (dpt,x), i]           [G,128,512]  per-core slice of x
      RB[(dpt,x), b*128+(dpt,y)]  [128,1280]   block-diag replicated blade
      W2[b, i, j]                 [9,512,512]  transposed weights
  * Phase 1 (per group, per i-chunk ic): one matmul per 512-col quad of RB:
      xbt[ic][i_loc, b*128+(dpt,y)] = sum_{(dpt,x)} xT[g,(dpt,x),ic*128+i_loc]
                                                    * RB[(dpt,x), col]
    which equals xb[pt,i,b,y] = sum_x x[pt,i,x]*blade[b,x,y].
  * Phase 2 (per group): accumulate 36 matmuls (b,ic) into one PSUM bank:
      out2[(dpt,y), j] += xbt[ic][:, b*128:(b+1)*128].T @ W2[b, ic-chunk, :]
    = out[pt, j, y].
  * Matmuls run in float32r (full-rate fp32 PE path; plain fp32 is 4x slower).
"""

from contextlib import ExitStack

import numpy as np

import concourse.bass as bass
import concourse.mybir as mybir
import concourse.tile as tile
from concourse import bacc
from concourse.bass_utils import run_bass_kernel_spmd

BATCH, NPTS, C, MV, BL = 8, 2048, 512, 16, 9
J = 512
N_CORES = 8
TOTAL_PTS = BATCH * NPTS            # 16384
GROUPS = TOTAL_PTS // 8             # 2048 groups of 8 points
GROUPS_PER_CORE = GROUPS // N_CORES  # 256

F32 = mybir.dt.float32
QUADS = ((0, 512), (512, 512), (1024, 256))  # RB column quads (last is padded)


def build_program(groups: int = GROUPS_PER_CORE, use_fp32r: bool = True) -> bass.Bass:
    nc = bacc.Bacc(trn_type="TRN2", target_bir_lowering=False, debug=False)
    mmdt = mybir.dt.float32r if use_fp32r else F32

    # fp32r inputs are pre-rounded (e8m11) on the host, so DMA straight into
    # fp32r tiles is numerically sound and walrus accepts it.
    xT_d = nc.dram_tensor("xT", [groups, 128, C], mmdt, kind="ExternalInput")
    w2_d = nc.dram_tensor("W2", [BL, C, J], mmdt, kind="ExternalInput")
    rb_d = nc.dram_tensor("RB", [128, 1280], mmdt, kind="ExternalInput")
    out_d = nc.dram_tensor("outT", [groups, 128, J], F32, kind="ExternalOutput")

    with tile.TileContext(nc) as tc, ExitStack() as ctx:
        const = ctx.enter_context(tc.tile_pool(name="const", bufs=1))
        xtp = ctx.enter_context(tc.tile_pool(name="xtp", bufs=3))
        xbp = ctx.enter_context(tc.tile_pool(name="xbp", bufs=8))
        osb = ctx.enter_context(tc.tile_pool(name="osb", bufs=3))
        ps1 = ctx.enter_context(tc.tile_pool(name="ps1", bufs=2, space="PSUM"))
        ps2 = ctx.enter_context(tc.tile_pool(name="ps2", bufs=2, space="PSUM"))

        w2t = {}
        for b in range(BL):
            for ic in range(4):
                t = const.tile([128, J], mmdt, tag=f"w2_{b}_{ic}")
                nc.sync.dma_start(out=t[:], in_=w2_d[b, ic * 128:(ic + 1) * 128, :])
                w2t[b, ic] = t
        rbt = const.tile([128, 1280], mmdt, tag="rb")
        nc.sync.dma_start(out=rbt[:], in_=rb_d[:])

        for g in range(groups):
            xt = xtp.tile([128, C], mmdt, tag="xt")
            nc.sync.dma_start(out=xt[:], in_=xT_d[g])

            xbts = []
            for ic in range(4):
                p1 = ps1.tile([128, 1280], F32, tag="p1")
                for c0, n in QUADS:
                    nc.tensor.matmul(
                        p1[:, c0:c0 + n],
                        xt[:, ic * 128:(ic + 1) * 128],
                        rbt[:, c0:c0 + n],
                        start=True, stop=True,
                    )
                xbt = xbp.tile([128, 1152], mmdt, tag="xbt")
                nc.vector.tensor_copy(xbt[:], p1[:, 0:1152])
                xbts.append(xbt)

            p2 = ps2.tile([128, J], F32, tag="p2")
            k = 0
            for b in range(BL):
                for ic in range(4):
                    nc.tensor.matmul(
                        p2[:],
                        xbts[ic][:, b * 128:(b + 1) * 128],
                        w2t[b, ic][:],
                        start=(k == 0), stop=(k == 35),
                    )
                    k += 1

            ot = osb.tile([128, J], F32, tag="osb")
            nc.scalar.copy(ot[:], p2[:])
            nc.sync.dma_start(out=out_d[g], in_=ot[:])

    nc.compile()
    return nc


def round_fp32r(a: np.ndarray) -> np.ndarray:
    """Round fp32 to the PE's fp32r format (e8m11): RNE to 11 mantissa bits."""
    u = np.ascontiguousarray(a, np.float32).view(np.uint32)
    lsb = (u >> 12) & 1
    u = ((u + 0x7FF + lsb) & np.uint32(0xFFFFF000)).astype(np.uint32)
    return u.view(np.float32)


def prep_inputs(x: np.ndarray, weights: np.ndarray, blade: np.ndarray):
    """Host-side layout prep. Returns (xT_all [GROUPS,128,C], W2 [BL,C,J], RB),
    all pre-rounded to fp32r."""
    x = np.ascontiguousarray(x, dtype=np.float32)
    xT = x.reshape(GROUPS, 8, C, MV).transpose(0, 1, 3, 2).reshape(GROUPS, 128, C)
    xT = round_fp32r(np.ascontiguousarray(xT))
    W2 = round_fp32r(np.ascontiguousarray(np.asarray(weights, np.float32).transpose(2, 1, 0)))
    RB = np.zeros((128, 1280), np.float32)
    bl = round_fp32r(np.asarray(blade, np.float32))
    for b in range(BL):
        for dpt in range(8):
            RB[dpt * 16:(dpt + 1) * 16, b * 128 + dpt * 16: b * 128 + (dpt + 1) * 16] = bl[b]
    return xT, W2, RB


def unprep_output(outT_all: np.ndarray) -> np.ndarray:
    """outT_all [GROUPS,128,J] -> out [BATCH,NPTS,J,MV]."""
    return np.ascontiguousarray(
        outT_all.reshape(GROUPS, 8, MV, J).transpose(0, 1, 3, 2)
        .reshape(BATCH, NPTS, J, MV)
    )


_NC_CACHE = {}


def _get_program():
    key = (GROUPS_PER_CORE, True)
    if key not in _NC_CACHE:
        _NC_CACHE[key] = build_program(*key)
    return _NC_CACHE[key]


def kernel(x: np.ndarray, weights: np.ndarray, blade: np.ndarray) -> np.ndarray:
    xT_all, W2, RB = prep_inputs(x, weights, blade)
    nc = _get_program()
    gpc = GROUPS_PER_CORE
    in_maps = [
        {"xT": xT_all[c * gpc:(c + 1) * gpc], "W2": W2, "RB": RB}
        for c in range(N_CORES)
    ]
    res = run_bass_kernel_spmd(nc, in_maps, list(range(N_CORES))).results
    outT_all = np.concatenate([res[c]["outT"] for c in range(N_CORES)], axis=0)
    return unprep_output(outT_all)


# revision 8
# speedup vs baseline: 24.8064x; 24.8064x over previous
"""Trainium2 Bass kernel for nn_EquiLinearLayer.

Computes  out[s,n,j,y] = sum_{i,x,b} weights[j,i,b] * blade[b,x,y] * x[s,n,i,x]
for x:[8,2048,512,16] f32, weights:[512,512,9] f32, blade:[9,16,16] f32.

Strategy (all-TensorE, two matmul phases, data-parallel over points):
  * 16384 points are split across 8 cores (2048 each), grouped in 8s
    (dpt in 0..7), 256 groups per core.
  * Host preps:
      xT[g, (dpt,x), i]           [G,128,512]  per-core slice of x
      RB[(dpt,x), b*128+(dpt,y)]  [128,1280]   block-diag replicated blade
      W2[b, i, j]                 [9,512,512]  transposed weights
  * Phase 1 (per group, per i-chunk ic): one matmul per 512-col quad of RB:
      xbt[ic][i_loc, b*128+(dpt,y)] = sum_{(dpt,x)} xT[g,(dpt,x),ic*128+i_loc]
                                                    * RB[(dpt,x), col]
    which equals xb[pt,i,b,y] = sum_x x[pt,i,x]*blade[b,x,y].
  * Phase 2 (per group): accumulate 36 matmuls (b,ic) into one PSUM bank:
      out2[(dpt,y), j] += xbt[ic][:, b*128:(b+1)*128].T @ W2[b, ic-chunk, :]
    = out[pt, j, y].
  * Matmuls run in float32r (full-rate fp32 PE path; plain fp32 is 4x slower).
"""

from contextlib import ExitStack

import numpy as np

import concourse.bass as bass
import concourse.mybir as mybir
import concourse.tile as tile
from concourse import bacc
from concourse.bass_utils import run_bass_kernel_spmd

BATCH, NPTS, C, MV, BL = 8, 2048, 512, 16, 9
J = 512
N_CORES = 8
TOTAL_PTS = BATCH * NPTS            # 16384
GROUPS = TOTAL_PTS // 8             # 2048 groups of 8 points
GROUPS_PER_CORE = GROUPS // N_CORES  # 256

F32 = mybir.dt.float32
QUADS = ((0, 512), (512, 512), (1024, 256))  # RB column quads (last is padded)


def build_program(groups: int = GROUPS_PER_CORE, use_fp32r: bool = True,
                  repeats: int = 1) -> bass.Bass:
    nc = bacc.Bacc(trn_type="TRN2", target_bir_lowering=False, debug=False)
    mmdt = mybir.dt.float32r if use_fp32r else F32

    # fp32r inputs are pre-rounded (e8m11) on the host, so DMA straight into
    # fp32r tiles is numerically sound and walrus accepts it.
    xT_d = nc.dram_tensor("xT", [groups, 128, C], mmdt, kind="ExternalInput")
    w2_d = nc.dram_tensor("W2", [BL, C, J], mmdt, kind="ExternalInput")
    rb_d = nc.dram_tensor("RB", [128, 1280], mmdt, kind="ExternalInput")
    out_d = nc.dram_tensor("outT", [groups, 128, J], F32, kind="ExternalOutput")

    with tile.TileContext(nc) as tc, ExitStack() as ctx:
        const = ctx.enter_context(tc.tile_pool(name="const", bufs=1))
        xtp = ctx.enter_context(tc.tile_pool(name="xtp", bufs=3))
        xbp = ctx.enter_context(tc.tile_pool(name="xbp", bufs=8))
        osb = ctx.enter_context(tc.tile_pool(name="osb", bufs=3))
        ps1 = ctx.enter_context(tc.tile_pool(name="ps1", bufs=2, space="PSUM"))
        ps2 = ctx.enter_context(tc.tile_pool(name="ps2", bufs=2, space="PSUM"))

        w2t = {}
        for b in range(BL):
            for ic in range(4):
                t = const.tile([128, J], mmdt, tag=f"w2_{b}_{ic}")
                nc.sync.dma_start(out=t[:], in_=w2_d[b, ic * 128:(ic + 1) * 128, :])
                w2t[b, ic] = t
        rbt = const.tile([128, 1280], mmdt, tag="rb")
        nc.sync.dma_start(out=rbt[:], in_=rb_d[:])

        for g in [g for _ in range(repeats) for g in range(groups)]:
            xt = xtp.tile([128, C], mmdt, tag="xt")
            nc.sync.dma_start(out=xt[:], in_=xT_d[g])

            xbts = []
            for ic in range(4):
                p1 = ps1.tile([128, 1280], F32, tag="p1")
                for c0, n in QUADS:
                    nc.tensor.matmul(
                        p1[:, c0:c0 + n],
                        xt[:, ic * 128:(ic + 1) * 128],
                        rbt[:, c0:c0 + n],
                        start=True, stop=True,
                    )
                xbt = xbp.tile([128, 1152], mmdt, tag="xbt")
                nc.vector.tensor_copy(xbt[:], p1[:, 0:1152])
                xbts.append(xbt)

            p2 = ps2.tile([128, J], F32, tag="p2")
            k = 0
            for b in range(BL):
                for ic in range(4):
                    nc.tensor.matmul(
                        p2[:],
                        xbts[ic][:, b * 128:(b + 1) * 128],
                        w2t[b, ic][:],
                        start=(k == 0), stop=(k == 35),
                    )
                    k += 1

            ot = osb.tile([128, J], F32, tag="osb")
            nc.scalar.copy(ot[:], p2[:])
            nc.sync.dma_start(out=out_d[g], in_=ot[:])

    nc.compile()
    return nc


def round_fp32r(a: np.ndarray) -> np.ndarray:
    """Round fp32 to the PE's fp32r format (e8m11): RNE to 11 mantissa bits."""
    u = np.ascontiguousarray(a, np.float32).view(np.uint32)
    lsb = (u >> 12) & 1
    u = ((u + 0x7FF + lsb) & np.uint32(0xFFFFF000)).astype(np.uint32)
    return u.view(np.float32)


def prep_inputs(x: np.ndarray, weights: np.ndarray, blade: np.ndarray):
    """Host-side layout prep. Returns (xT_all [GROUPS,128,C], W2 [BL,C,J], RB),
    all pre-rounded to fp32r."""
    x = np.ascontiguousarray(x, dtype=np.float32)
    xT = x.reshape(GROUPS, 8, C, MV).transpose(0, 1, 3, 2).reshape(GROUPS, 128, C)
    xT = round_fp32r(np.ascontiguousarray(xT))
    W2 = round_fp32r(np.ascontiguousarray(np.asarray(weights, np.float32).transpose(2, 1, 0)))
    RB = np.zeros((128, 1280), np.float32)
    bl = round_fp32r(np.asarray(blade, np.float32))
    for b in range(BL):
        for dpt in range(8):
            RB[dpt * 16:(dpt + 1) * 16, b * 128 + dpt * 16: b * 128 + (dpt + 1) * 16] = bl[b]
    return xT, W2, RB


def unprep_output(outT_all: np.ndarray) -> np.ndarray:
    """outT_all [GROUPS,128,J] -> out [BATCH,NPTS,J,MV]."""
    return np.ascontiguousarray(
        outT_all.reshape(GROUPS, 8, MV, J).transpose(0, 1, 3, 2)
        .reshape(BATCH, NPTS, J, MV)
    )


_NC_CACHE = {}


def _get_program():
    key = (GROUPS_PER_CORE, True)
    if key not in _NC_CACHE:
        _NC_CACHE[key] = build_program(*key)
    return _NC_CACHE[key]


def kernel(x: np.ndarray, weights: np.ndarray, blade: np.ndarray) -> np.ndarray:
    xT_all, W2, RB = prep_inputs(x, weights, blade)
    nc = _get_program()
    gpc = GROUPS_PER_CORE
    in_maps = [
        {"xT": xT_all[c * gpc:(c + 1) * gpc], "W2": W2, "RB": RB}
        for c in range(N_CORES)
    ]
    res = run_bass_kernel_spmd(nc, in_maps, list(range(N_CORES))).results
    outT_all = np.concatenate([res[c]["outT"] for c in range(N_CORES)], axis=0)
    return unprep_output(outT_all)


# revision 9
# speedup vs baseline: 26.2147x; 1.0568x over previous
"""Trainium2 Bass kernel for nn_EquiLinearLayer.

Computes  out[s,n,j,y] = sum_{i,x,b} weights[j,i,b] * blade[b,x,y] * x[s,n,i,x]
for x:[8,2048,512,16] f32, weights:[512,512,9] f32, blade:[9,16,16] f32.

Strategy (all-TensorE, two matmul phases, data-parallel over points):
  * 16384 points are split across 8 cores (2048 each), grouped in 8s
    (dpt in 0..7), 256 groups per core.
  * Host preps:
      xT[g, (dpt,x), i]           [G,128,512]  per-core slice of x
      RB[(dpt,x), b*128+(dpt,y)]  [128,1280]   block-diag replicated blade
      W2[b, i, j]                 [9,512,512]  transposed weights
  * Phase 1 (per group, per i-chunk ic): one matmul per 512-col quad of RB:
      xbt[ic][i_loc, b*128+(dpt,y)] = sum_{(dpt,x)} xT[g,(dpt,x),ic*128+i_loc]
                                                    * RB[(dpt,x), col]
    which equals xb[pt,i,b,y] = sum_x x[pt,i,x]*blade[b,x,y].
  * Phase 2 (per group): accumulate 36 matmuls (b,ic) into one PSUM bank:
      out2[(dpt,y), j] += xbt[ic][:, b*128:(b+1)*128].T @ W2[b, ic-chunk, :]
    = out[pt, j, y].
  * Matmuls run in float32r (full-rate fp32 PE path; plain fp32 is 4x slower).
"""

from contextlib import ExitStack

import numpy as np

import concourse.bass as bass
import concourse.mybir as mybir
import concourse.tile as tile
from concourse import bacc
from concourse.bass_utils import run_bass_kernel_spmd

BATCH, NPTS, C, MV, BL = 8, 2048, 512, 16, 9
J = 512
N_CORES = 8
TOTAL_PTS = BATCH * NPTS            # 16384
GROUPS = TOTAL_PTS // 8             # 2048 groups of 8 points
GROUPS_PER_CORE = GROUPS // N_CORES  # 256

F32 = mybir.dt.float32
QUADS = ((0, 512), (512, 512), (1024, 256))  # RB column quads (last is padded)


def build_program(groups: int = GROUPS_PER_CORE, use_fp32r: bool = True,
                  repeats: int = 1) -> bass.Bass:
    nc = bacc.Bacc(trn_type="TRN2", target_bir_lowering=False, debug=False)
    mmdt = mybir.dt.float32r if use_fp32r else F32

    # fp32r inputs are pre-rounded (e8m11) on the host, so DMA straight into
    # fp32r tiles is numerically sound and walrus accepts it.
    xT_d = nc.dram_tensor("xT", [groups, 128, C], mmdt, kind="ExternalInput")
    w2_d = nc.dram_tensor("W2", [BL, C, J], mmdt, kind="ExternalInput")
    rb_d = nc.dram_tensor("RB", [128, 1280], mmdt, kind="ExternalInput")
    out_d = nc.dram_tensor("outT", [groups, 128, J], F32, kind="ExternalOutput")

    with tile.TileContext(nc) as tc, ExitStack() as ctx:
        const = ctx.enter_context(tc.tile_pool(name="const", bufs=1))
        xtp = ctx.enter_context(tc.tile_pool(name="xtp", bufs=3))
        xbp = ctx.enter_context(tc.tile_pool(name="xbp", bufs=8))
        osb = ctx.enter_context(tc.tile_pool(name="osb", bufs=3))
        ps1 = ctx.enter_context(tc.tile_pool(name="ps1", bufs=2, space="PSUM"))
        ps2 = ctx.enter_context(tc.tile_pool(name="ps2", bufs=2, space="PSUM"))

        w2t = {}
        for b in range(BL):
            for ic in range(4):
                t = const.tile([128, J], mmdt, tag=f"w2_{b}_{ic}")
                nc.sync.dma_start(out=t[:], in_=w2_d[b, ic * 128:(ic + 1) * 128, :])
                w2t[b, ic] = t
        rbt = const.tile([128, 1280], mmdt, tag="rb")
        nc.sync.dma_start(out=rbt[:], in_=rb_d[:])

        def ph1_half(xt, ics):
            """Phase-1 matmuls + psum evac for the given i-chunks."""
            out = []
            for ic in ics:
                p1 = ps1.tile([128, 1280], F32, tag="p1")
                for c0, n in QUADS:
                    nc.tensor.matmul(
                        p1[:, c0:c0 + n],
                        xt[:, ic * 128:(ic + 1) * 128],
                        rbt[:, c0:c0 + n],
                        start=True, stop=True,
                    )
                xbt = xbp.tile([128, 1152], mmdt, tag="xbt")
                nc.vector.tensor_copy(xbt[:], p1[:, 0:1152])
                out.append(xbt)
            return out

        def ph2(g, xbts):
            p2 = ps2.tile([128, J], F32, tag="p2")
            k = 0
            for ic in range(4):
                for b in range(BL):
                    nc.tensor.matmul(
                        p2[:],
                        xbts[ic][:, b * 128:(b + 1) * 128],
                        w2t[b, ic][:],
                        start=(k == 0), stop=(k == 35),
                    )
                    k += 1
            ot = osb.tile([128, J], F32, tag="osb")
            nc.scalar.copy(ot[:], p2[:])
            nc.sync.dma_start(out=out_d[g], in_=ot[:])

        # Software pipelining: phase-1 of group g is emitted in two halves
        # around phase-2 of group g-1, so phase-1's psum-slot waits (evac on
        # DVE) overlap phase-2's 9us matmul stream instead of stalling PE.
        pending = None
        for g in [g for _ in range(repeats) for g in range(groups)]:
            xt = xtp.tile([128, C], mmdt, tag="xt")
            nc.sync.dma_start(out=xt[:], in_=xT_d[g])
            first = ph1_half(xt, (0, 1))
            if pending is not None:
                ph2(*pending)
            second = ph1_half(xt, (2, 3))
            pending = (g, first + second)
        if pending is not None:
            ph2(*pending)

    nc.compile()
    return nc


def round_fp32r(a: np.ndarray) -> np.ndarray:
    """Round fp32 to the PE's fp32r format (e8m11): RNE to 11 mantissa bits."""
    u = np.ascontiguousarray(a, np.float32).view(np.uint32)
    lsb = (u >> 12) & 1
    u = ((u + 0x7FF + lsb) & np.uint32(0xFFFFF000)).astype(np.uint32)
    return u.view(np.float32)


def prep_inputs(x: np.ndarray, weights: np.ndarray, blade: np.ndarray):
    """Host-side layout prep. Returns (xT_all [GROUPS,128,C], W2 [BL,C,J], RB),
    all pre-rounded to fp32r."""
    x = np.ascontiguousarray(x, dtype=np.float32)
    xT = x.reshape(GROUPS, 8, C, MV).transpose(0, 1, 3, 2).reshape(GROUPS, 128, C)
    xT = round_fp32r(np.ascontiguousarray(xT))
    W2 = round_fp32r(np.ascontiguousarray(np.asarray(weights, np.float32).transpose(2, 1, 0)))
    RB = np.zeros((128, 1280), np.float32)
    bl = round_fp32r(np.asarray(blade, np.float32))
    for b in range(BL):
        for dpt in range(8):
            RB[dpt * 16:(dpt + 1) * 16, b * 128 + dpt * 16: b * 128 + (dpt + 1) * 16] = bl[b]
    return xT, W2, RB


def unprep_output(outT_all: np.ndarray) -> np.ndarray:
    """outT_all [GROUPS,128,J] -> out [BATCH,NPTS,J,MV]."""
    return np.ascontiguousarray(
        outT_all.reshape(GROUPS, 8, MV, J).transpose(0, 1, 3, 2)
        .reshape(BATCH, NPTS, J, MV)
    )


_NC_CACHE = {}


def _get_program():
    key = (GROUPS_PER_CORE, True)
    if key not in _NC_CACHE:
        _NC_CACHE[key] = build_program(*key)
    return _NC_CACHE[key]


def kernel(x: np.ndarray, weights: np.ndarray, blade: np.ndarray) -> np.ndarray:
    xT_all, W2, RB = prep_inputs(x, weights, blade)
    nc = _get_program()
    gpc = GROUPS_PER_CORE
    in_maps = [
        {"xT": xT_all[c * gpc:(c + 1) * gpc], "W2": W2, "RB": RB}
        for c in range(N_CORES)
    ]
    res = run_bass_kernel_spmd(nc, in_maps, list(range(N_CORES))).results
    outT_all = np.concatenate([res[c]["outT"] for c in range(N_CORES)], axis=0)
    return unprep_output(outT_all)


# revision 10
# speedup vs baseline: 29.7480x; 1.1348x over previous
"""Trainium2 Bass kernel for nn_EquiLinearLayer.

Computes  out[s,n,j,y] = sum_{i,x,b} weights[j,i,b] * blade[b,x,y] * x[s,n,i,x]
for x:[8,2048,512,16] f32, weights:[512,512,9] f32, blade:[9,16,16] f32.

Strategy (all-TensorE, two matmul phases, data-parallel over points):
  * 16384 points are split across 8 cores (2048 each), grouped in 8s
    (dpt in 0..7), 256 groups per core.
  * Host preps:
      xT[g, (dpt,x), i]           [G,128,512]  per-core slice of x
      RB[(dpt,x), b*128+(dpt,y)]  [128,1280]   block-diag replicated blade
      W2[b, i, j]                 [9,512,512]  transposed weights
  * Phase 1 (per group, per i-chunk ic): one matmul per 512-col quad of RB:
      xbt[ic][i_loc, b*128+(dpt,y)] = sum_{(dpt,x)} xT[g,(dpt,x),ic*128+i_loc]
                                                    * RB[(dpt,x), col]
    which equals xb[pt,i,b,y] = sum_x x[pt,i,x]*blade[b,x,y].
  * Phase 2 (per group): accumulate 36 matmuls (b,ic) into one PSUM bank:
      out2[(dpt,y), j] += xbt[ic][:, b*128:(b+1)*128].T @ W2[b, ic-chunk, :]
    = out[pt, j, y].
  * Matmuls run in float32r (full-rate fp32 PE path; plain fp32 is 4x slower).
"""

from contextlib import ExitStack

import numpy as np

import concourse.bass as bass
import concourse.mybir as mybir
import concourse.tile as tile
from concourse import bacc
from concourse.bass_utils import run_bass_kernel_spmd

BATCH, NPTS, C, MV, BL = 8, 2048, 512, 16, 9
J = 512
N_CORES = 8
TOTAL_PTS = BATCH * NPTS            # 16384
GROUPS = TOTAL_PTS // 8             # 2048 groups of 8 points
GROUPS_PER_CORE = GROUPS // N_CORES  # 256

F32 = mybir.dt.float32
QUADS = ((0, 512), (512, 512), (1024, 256))  # RB column quads (last is padded)


def build_program(groups: int = GROUPS_PER_CORE, use_fp32r: bool = True,
                  repeats: int = 1) -> bass.Bass:
    nc = bacc.Bacc(trn_type="TRN2", target_bir_lowering=False, debug=False)
    mmdt = mybir.dt.float32r if use_fp32r else F32

    # fp32r inputs are pre-rounded (e8m11) on the host, so DMA straight into
    # fp32r tiles is numerically sound and walrus accepts it.
    xT_d = nc.dram_tensor("xT", [groups, 128, C], mmdt, kind="ExternalInput")
    w2_d = nc.dram_tensor("W2", [BL, C, J], mmdt, kind="ExternalInput")
    rb_d = nc.dram_tensor("RB", [128, 1280], mmdt, kind="ExternalInput")
    out_d = nc.dram_tensor("outT", [groups, 128, J], F32, kind="ExternalOutput")

    with tile.TileContext(nc) as tc, ExitStack() as ctx:
        const = ctx.enter_context(tc.tile_pool(name="const", bufs=1))
        xtp = ctx.enter_context(tc.tile_pool(name="xtp", bufs=3))
        xbp = ctx.enter_context(tc.tile_pool(name="xbp", bufs=8))
        osb = ctx.enter_context(tc.tile_pool(name="osb", bufs=3))
        ps1 = ctx.enter_context(tc.tile_pool(name="ps1", bufs=2, space="PSUM"))
        ps2 = ctx.enter_context(tc.tile_pool(name="ps2", bufs=2, space="PSUM"))

        w2t = {}
        for b in range(BL):
            for ic in range(4):
                t = const.tile([128, J], mmdt, tag=f"w2_{b}_{ic}")
                nc.sync.dma_start(out=t[:], in_=w2_d[b, ic * 128:(ic + 1) * 128, :])
                w2t[b, ic] = t
        rbt = const.tile([128, 1280], mmdt, tag="rb")
        nc.sync.dma_start(out=rbt[:], in_=rb_d[:])

        def ph1_half(xt, ics):
            """Phase-1 matmuls + psum evac for the given i-chunks."""
            out = []
            for ic in ics:
                p1 = ps1.tile([128, 1280], F32, tag="p1")
                for c0, n in QUADS:
                    nc.tensor.matmul(
                        p1[:, c0:c0 + n],
                        xt[:, ic * 128:(ic + 1) * 128],
                        rbt[:, c0:c0 + n],
                        start=True, stop=True,
                    )
                xbt = xbp.tile([128, 1152], mmdt, tag="xbt")
                nc.vector.tensor_copy(xbt[:], p1[:, 0:1152])
                out.append(xbt)
            return out

        def ph2(g, xbts):
            p2 = ps2.tile([128, J], F32, tag="p2")
            k = 0
            for ic in range(4):
                for b in range(BL):
                    nc.tensor.matmul(
                        p2[:],
                        xbts[ic][:, b * 128:(b + 1) * 128],
                        w2t[b, ic][:],
                        start=(k == 0), stop=(k == 35),
                    )
                    k += 1
            ot = osb.tile([128, J], F32, tag="osb")
            nc.scalar.copy(ot[:], p2[:])
            nc.sync.dma_start(out=out_d[g], in_=ot[:])

        # Software pipelining: phase-1 of group g is emitted in two halves
        # around phase-2 of group g-1, so phase-1's psum-slot waits (evac on
        # DVE) overlap phase-2's 9us matmul stream instead of stalling PE.
        pending = None
        for g in [g for _ in range(repeats) for g in range(groups)]:
            xt = xtp.tile([128, C], mmdt, tag="xt")
            nc.sync.dma_start(out=xt[:], in_=xT_d[g])
            first = ph1_half(xt, (0, 1))
            if pending is not None:
                ph2(*pending)
            second = ph1_half(xt, (2, 3))
            pending = (g, first + second)
        if pending is not None:
            ph2(*pending)

    nc.compile()
    return nc


def round_fp32r(a: np.ndarray) -> np.ndarray:
    """Round fp32 to the PE's fp32r format (e8m11): RNE to 11 mantissa bits."""
    u = np.ascontiguousarray(a, np.float32).view(np.uint32)
    lsb = (u >> 12) & 1
    u = ((u + 0x7FF + lsb) & np.uint32(0xFFFFF000)).astype(np.uint32)
    return u.view(np.float32)


def prep_inputs(x: np.ndarray, weights: np.ndarray, blade: np.ndarray):
    """Host-side layout prep. Returns (xT_all [GROUPS,128,C], W2 [BL,C,J], RB),
    all pre-rounded to fp32r."""
    x = np.ascontiguousarray(x, dtype=np.float32)
    xT = x.reshape(GROUPS, 8, C, MV).transpose(0, 1, 3, 2).reshape(GROUPS, 128, C)
    xT = round_fp32r(np.ascontiguousarray(xT))
    W2 = round_fp32r(np.ascontiguousarray(np.asarray(weights, np.float32).transpose(2, 1, 0)))
    RB = np.zeros((128, 1280), np.float32)
    bl = round_fp32r(np.asarray(blade, np.float32))
    for b in range(BL):
        for dpt in range(8):
            RB[dpt * 16:(dpt + 1) * 16, b * 128 + dpt * 16: b * 128 + (dpt + 1) * 16] = bl[b]
    return xT, W2, RB


def unprep_output(outT_all: np.ndarray) -> np.ndarray:
    """outT_all [GROUPS,128,J] -> out [BATCH,NPTS,J,MV]."""
    return np.ascontiguousarray(
        outT_all.reshape(GROUPS, 8, MV, J).transpose(0, 1, 3, 2)
        .reshape(BATCH, NPTS, J, MV)
    )


_NC_CACHE = {}


def _get_program():
    key = (GROUPS_PER_CORE, True)
    if key not in _NC_CACHE:
        _NC_CACHE[key] = build_program(*key)
    return _NC_CACHE[key]


def kernel(x: np.ndarray, weights: np.ndarray, blade: np.ndarray) -> np.ndarray:
    xT_all, W2, RB = prep_inputs(x, weights, blade)
    nc = _get_program()
    gpc = GROUPS_PER_CORE
    in_maps = [
        {"xT": xT_all[c * gpc:(c + 1) * gpc], "W2": W2, "RB": RB}
        for c in range(N_CORES)
    ]
    try:
        res = run_bass_kernel_spmd(nc, in_maps, list(range(N_CORES))).results
    except Exception:
        # Transient NRT/axon faults (e.g. NRT_EXEC_UNIT_UNRECOVERABLE) have
        # been observed across rapid successive sessions; retry once.
        import time as _time
        _time.sleep(10)
        res = run_bass_kernel_spmd(nc, in_maps, list(range(N_CORES))).results
    outT_all = np.concatenate([res[c]["outT"] for c in range(N_CORES)], axis=0)
    return unprep_output(outT_all)


# revision 11
# speedup vs baseline: 36.5452x; 1.2285x over previous
"""Trainium2 Bass kernel for nn_EquiLinearLayer.

Computes  out[s,n,j,y] = sum_{i,x,b} weights[j,i,b] * blade[b,x,y] * x[s,n,i,x]
for x:[8,2048,512,16] f32, weights:[512,512,9] f32, blade:[9,16,16] f32.

Strategy (all-TensorE, two matmul phases, data-parallel over points):
  * 16384 points are split across 8 cores (2048 each), grouped in 8s
    (dpt in 0..7), 256 groups per core.
  * Host preps:
      xT[g, (dpt,x), i]           [G,128,512]  per-core slice of x
      RB[(dpt,x), b*128+(dpt,y)]  [128,1280]   block-diag replicated blade
      W2[b, i, j]                 [9,512,512]  transposed weights
  * Phase 1 (per group, per i-chunk ic): one matmul per 512-col quad of RB:
      xbt[ic][i_loc, b*128+(dpt,y)] = sum_{(dpt,x)} xT[g,(dpt,x),ic*128+i_loc]
                                                    * RB[(dpt,x), col]
    which equals xb[pt,i,b,y] = sum_x x[pt,i,x]*blade[b,x,y].
  * Phase 2 (per group): accumulate 36 matmuls (b,ic) into one PSUM bank:
      out2[(dpt,y), j] += xbt[ic][:, b*128:(b+1)*128].T @ W2[b, ic-chunk, :]
    = out[pt, j, y].
  * Matmuls run in float32r (full-rate fp32 PE path; plain fp32 is 4x slower).
"""

from contextlib import ExitStack

import numpy as np

import concourse.bass as bass
import concourse.mybir as mybir
import concourse.tile as tile
from concourse import bacc
from concourse.bass_utils import run_bass_kernel_spmd

BATCH, NPTS, C, MV, BL = 8, 2048, 512, 16, 9
J = 512
N_CORES = 8
TOTAL_PTS = BATCH * NPTS            # 16384
GROUPS = TOTAL_PTS // 8             # 2048 groups of 8 points
GROUPS_PER_CORE = GROUPS // N_CORES  # 256

F32 = mybir.dt.float32
QUADS = ((0, 512), (512, 512), (1024, 256))  # RB column quads (last is padded)


def build_program(groups: int = GROUPS_PER_CORE, use_fp32r: bool = True,
                  repeats: int = 1) -> bass.Bass:
    nc = bacc.Bacc(trn_type="TRN2", target_bir_lowering=False, debug=False)
    mmdt = mybir.dt.float32r if use_fp32r else F32

    # fp32r inputs are pre-rounded (e8m11) on the host, so DMA straight into
    # fp32r tiles is numerically sound and walrus accepts it.
    xT_d = nc.dram_tensor("xT", [groups, 128, C], mmdt, kind="ExternalInput")
    w2_d = nc.dram_tensor("W2", [BL, C, J], mmdt, kind="ExternalInput")
    rb_d = nc.dram_tensor("RB", [128, 1280], mmdt, kind="ExternalInput")
    out_d = nc.dram_tensor("outT", [groups, 128, J], F32, kind="ExternalOutput")

    with tile.TileContext(nc) as tc, ExitStack() as ctx:
        const = ctx.enter_context(tc.tile_pool(name="const", bufs=1))
        xtp = ctx.enter_context(tc.tile_pool(name="xtp", bufs=4))
        xbp = ctx.enter_context(tc.tile_pool(name="xbp", bufs=12))
        osb = ctx.enter_context(tc.tile_pool(name="osb", bufs=4))
        ps1 = ctx.enter_context(tc.tile_pool(name="ps1", bufs=2, space="PSUM"))
        ps2 = ctx.enter_context(tc.tile_pool(name="ps2", bufs=2, space="PSUM"))

        w2t = {}
        for b in range(BL):
            for ic in range(4):
                t = const.tile([128, J], mmdt, tag=f"w2_{b}_{ic}")
                nc.sync.dma_start(out=t[:], in_=w2_d[b, ic * 128:(ic + 1) * 128, :])
                w2t[b, ic] = t
        rbt = const.tile([128, 1280], mmdt, tag="rb")
        nc.sync.dma_start(out=rbt[:], in_=rb_d[:])

        def ph1_half(xt, ics):
            """Phase-1 matmuls + psum evac for the given i-chunks."""
            out = []
            for ic in ics:
                p1 = ps1.tile([128, 1280], F32, tag="p1")
                for c0, n in QUADS:
                    nc.tensor.matmul(
                        p1[:, c0:c0 + n],
                        xt[:, ic * 128:(ic + 1) * 128],
                        rbt[:, c0:c0 + n],
                        start=True, stop=True,
                    )
                xbt = xbp.tile([128, 1152], mmdt, tag="xbt")
                nc.vector.tensor_copy(xbt[:], p1[:, 0:1152])
                out.append(xbt)
            return out

        def ph2(g, xbts):
            p2 = ps2.tile([128, J], F32, tag="p2")
            k = 0
            for ic in range(4):
                for b in range(BL):
                    nc.tensor.matmul(
                        p2[:],
                        xbts[ic][:, b * 128:(b + 1) * 128],
                        w2t[b, ic][:],
                        start=(k == 0), stop=(k == 35),
                    )
                    k += 1
            ot = osb.tile([128, J], F32, tag="osb")
            nc.scalar.copy(ot[:], p2[:])
            nc.sync.dma_start(out=out_d[g], in_=ot[:])

        # Software pipelining: phase-1 of group g is emitted in two halves
        # around phase-2 of group g-1, so phase-1's psum-slot waits (evac on
        # DVE) overlap phase-2's 9us matmul stream instead of stalling PE.
        pending = None
        for g in [g for _ in range(repeats) for g in range(groups)]:
            xt = xtp.tile([128, C], mmdt, tag="xt")
            nc.sync.dma_start(out=xt[:], in_=xT_d[g])
            first = ph1_half(xt, (0, 1))
            if pending is not None:
                ph2(*pending)
            second = ph1_half(xt, (2, 3))
            pending = (g, first + second)
        if pending is not None:
            ph2(*pending)

    nc.compile()
    return nc


def round_fp32r(a: np.ndarray) -> np.ndarray:
    """Round fp32 to the PE's fp32r format (e8m11): RNE to 11 mantissa bits."""
    u = np.ascontiguousarray(a, np.float32).view(np.uint32)
    lsb = (u >> 12) & 1
    u = ((u + 0x7FF + lsb) & np.uint32(0xFFFFF000)).astype(np.uint32)
    return u.view(np.float32)


def prep_inputs(x: np.ndarray, weights: np.ndarray, blade: np.ndarray):
    """Host-side layout prep. Returns (xT_all [GROUPS,128,C], W2 [BL,C,J], RB),
    all pre-rounded to fp32r."""
    x = np.ascontiguousarray(x, dtype=np.float32)
    xT = x.reshape(GROUPS, 8, C, MV).transpose(0, 1, 3, 2).reshape(GROUPS, 128, C)
    xT = round_fp32r(np.ascontiguousarray(xT))
    W2 = round_fp32r(np.ascontiguousarray(np.asarray(weights, np.float32).transpose(2, 1, 0)))
    RB = np.zeros((128, 1280), np.float32)
    bl = round_fp32r(np.asarray(blade, np.float32))
    for b in range(BL):
        for dpt in range(8):
            RB[dpt * 16:(dpt + 1) * 16, b * 128 + dpt * 16: b * 128 + (dpt + 1) * 16] = bl[b]
    return xT, W2, RB


def unprep_output(outT_all: np.ndarray) -> np.ndarray:
    """outT_all [GROUPS,128,J] -> out [BATCH,NPTS,J,MV]."""
    return np.ascontiguousarray(
        outT_all.reshape(GROUPS, 8, MV, J).transpose(0, 1, 3, 2)
        .reshape(BATCH, NPTS, J, MV)
    )


_NC_CACHE = {}


def _get_program():
    key = (GROUPS_PER_CORE, True)
    if key not in _NC_CACHE:
        _NC_CACHE[key] = build_program(*key)
    return _NC_CACHE[key]


def kernel(x: np.ndarray, weights: np.ndarray, blade: np.ndarray) -> np.ndarray:
    xT_all, W2, RB = prep_inputs(x, weights, blade)
    nc = _get_program()
    gpc = GROUPS_PER_CORE
    in_maps = [
        {"xT": xT_all[c * gpc:(c + 1) * gpc], "W2": W2, "RB": RB}
        for c in range(N_CORES)
    ]
    try:
        res = run_bass_kernel_spmd(nc, in_maps, list(range(N_CORES))).results
    except Exception:
        # Transient NRT/axon faults (e.g. NRT_EXEC_UNIT_UNRECOVERABLE) have
        # been observed across rapid successive sessions; retry once.
        import time as _time
        _time.sleep(10)
        res = run_bass_kernel_spmd(nc, in_maps, list(range(N_CORES))).results
    outT_all = np.concatenate([res[c]["outT"] for c in range(N_CORES)], axis=0)
    return unprep_output(outT_all)
